# revision 1
# baseline (speedup 1.0000x reference)
"""Causal self-attention (B=2, T=2048, C=1024, H=16) on 8 TRN2 NeuronCores.

Sharding: 8 cores = 2 batches x 4 head-groups (4 heads each).
Each core computes qkv for its heads, causal attention, and a partial
output projection; the host sums the 4 partial projections per batch.

All matmuls run in float32r (TF32-like) at 1 cycle/row.

Layouts (per core):
  xT   [C, T]        x^T, streamed in [128, 512] slices
  wT   [C, 768]      qkv weight slice, pre-transposed; column order
                     [q01|k01|v01|q23|k23|v23] so pair-0 weights land first
  qkvT [6][128, T]   m0,m1 = q^T (heads 01, 23); m2,m3 = k^T; m4,m5 = v^T
  vaug [2][128, 2080] per head pair: 16 k-tile groups of 130 cols =
                     [v_h0 (64) | ones | v_h1 (64) | ones]
  attention in scoresT layout: partition = k, free = q. exp on ScalarE
  handles both heads of a pair in one instruction (2-bank PSUM tile).
  av^T accumulated via matmul with v_aug (ones column -> softmax sums).

Emission is interleaved per q-block (qkv chains for block n, then
attention for q-block n) so all engines ramp up early.
"""

import numpy as np

import concourse.bass as bass
import concourse.mybir as mybir
import concourse.tile as tile
from concourse import bacc, bass_utils
F32 = mybir.dt.float32
F32R = mybir.dt.float32r
AF = mybir.ActivationFunctionType

B = 2
T = 2048
C = 1024
D = 64
N_CORES = 8
HG = 4            # heads per core
CG = HG * D       # 256 y-columns per core
P = 128
TQ = 512          # q block width
NKT = T // P      # 16 k tiles
NQB = T // TQ     # 4 q blocks
NCT = C // P      # 8 contraction tiles for qkv
NM = 3 * CG // P  # 6 output m-tiles for qkvT

# logical qkvT m-tile -> physical column slot in wT (pair-0 tiles first)
WCOL = {0: 0, 2: 1, 4: 2, 1: 3, 3: 4, 5: 5}

_cached = {}


def _build_nc():
    nc = bacc.Bacc("TRN2", target_bir_lowering=False, debug=False,
                   num_devices=N_CORES)
    xT = nc.dram_tensor("xT", [C, T], F32, kind="ExternalInput")
    wT = nc.dram_tensor("wT", [C, 3 * CG], F32, kind="ExternalInput")
    pT = nc.dram_tensor("pT", [CG, C], F32, kind="ExternalInput")
    cst = nc.dram_tensor("cst", [P, 576], F32, kind="ExternalInput")
    out = nc.dram_tensor("out", [T, C], F32, kind="ExternalOutput")

    with tile.TileContext(nc) as tc:
        with (
            tc.tile_pool(name="const", bufs=1) as const,
            tc.tile_pool(name="persist", bufs=1) as persist,
            tc.tile_pool(name="xw", bufs=2) as xw,
            tc.tile_pool(name="wpool", bufs=1) as wpool,
            tc.tile_pool(name="esb", bufs=3) as esb,
            tc.tile_pool(name="small", bufs=2) as small,
            tc.tile_pool(name="psSC", bufs=2, space="PSUM") as psSC,
            tc.tile_pool(name="psMM", bufs=2, space="PSUM") as psMM,
            tc.tile_pool(name="psAv", bufs=1, space="PSUM") as psAv,
        ):
            # -------- constants (host-prepared, one DMA) --------
            # layout: [ident 128 | tri2 256 | ones 64 | sel 128]
            cstt = const.tile([P, 576], F32R, tag="cstt")
            nc.sync.dma_start(cstt[:], cst[:].bitcast(F32R))
            ident = cstt[:, 0:P]
            tri2 = cstt[:, P:3 * P]
            onesf = cstt[:, 3 * P:3 * P + D]

            # ---------------- persistent tensors ----------------
            qkvT = [persist.tile([P, T], F32R, tag=f"qkvT{m}", name=f"qkvT{m}")
                    for m in range(NM)]
            vaug = [persist.tile([P, NKT * 2 * (D + 1)], F32R,
                                 tag=f"vaug{j}", name=f"vaug{j}")
                    for j in range(2)]
            yT = [persist.tile([P, T], F32R, tag=f"yT{j}", name=f"yT{j}")
                  for j in range(2)]
            wp = [persist.tile([P, C], F32R, tag=f"wp{j}", name=f"wp{j}")
                  for j in range(2)]

            # ---------------- input DMAs ----------------
            xs = {}
            wt = []
            for n in range(NQB):
                for c in range(NCT):
                    t0 = n * TQ
                    xs[(c, n)] = xw.tile([P, TQ], F32R, tag=f"x{c}",
                                         name=f"x{c}_{n}")
                    if n == 0:
                        # interleave x and W per c so chain c-steps start early
                        w_ = wpool.tile([P, 3 * CG], F32R, tag=f"w{c}",
                                        name=f"w{c}")
                        wt.append(w_)
                        nc.sync.dma_start(
                            w_[:, :384],
                            wT[c * P:(c + 1) * P, :384].bitcast(F32R))
                        nc.sync.dma_start(
                            xs[(c, n)][:],
                            xT[c * P:(c + 1) * P, t0:t0 + TQ].bitcast(F32R))
                        nc.sync.dma_start(
                            w_[:, 384:],
                            wT[c * P:(c + 1) * P, 384:].bitcast(F32R))
                    else:
                        nc.sync.dma_start(
                            xs[(c, n)][:],
                            xT[c * P:(c + 1) * P, t0:t0 + TQ].bitcast(F32R))
                if n == 0:
                    for j in range(2):
                        nc.sync.dma_start(
                            wp[j][:],
                            pT[j * P:(j + 1) * P, :].bitcast(F32R))

            # -------- software-pipelined emission --------
            # PE executes its stream in order, so QKV chains / v-transposes
            # for block n+1 and proj matmuls for block qb-1 are interleaved
            # into the (ACT-bound) attention k-loop of block qb.
            vrs = [vaug[j].rearrange("p (k g x) -> p k g x", k=NKT, g=2)
                   for j in range(2)]

            def emit_chain(n, m):
                mc = WCOL[m]
                ps = psMM.tile([P, TQ], F32, tag="mm", name=f"psA_{n}_{m}")
                for c in range(NCT):
                    nc.tensor.matmul(
                        ps[:],
                        wt[c][:, mc * P:(mc + 1) * P],
                        xs[(c, n)][:],
                        start=(c == 0), stop=(c == NCT - 1),
                    )
                nc.vector.tensor_copy(qkvT[m][:, n * TQ:(n + 1) * TQ], ps[:])

            def emit_pt(j, kt):
                pt = psMM.tile([P, P], F32R, tag="mm", name=f"pt_{j}_{kt}")
                nc.tensor.transpose(
                    pt[:], qkvT[4 + j][:, kt * P:(kt + 1) * P], ident)
                nc.vector.tensor_copy(
                    vrs[j][:, kt, :, :D],
                    pt[:].rearrange("p (g x) -> p g x", g=2))

            def emit_ones(n, j):
                nc.vector.tensor_copy(
                    vaug[j][:, D + 65 * 8 * n:130 * (4 * n + 4):D + 1],
                    onesf[:, :8])

            def emit_pp(tb, oh, engine):
                pp = psMM.tile([P, TQ], F32, tag="mm", name=f"pp_{tb}_{oh}")
                for cc in range(2):
                    nc.tensor.matmul(
                        pp[:],
                        yT[cc][:, tb * P:(tb + 1) * P],
                        wp[cc][:, oh * TQ:(oh + 1) * TQ],
                        start=(cc == 0), stop=(cc == 1),
                    )
                ob = esb.tile([P, TQ], F32, tag="ob", name=f"ob_{tb}_{oh}")
                if engine == "act":
                    nc.scalar.copy(ob[:], pp[:])
                else:
                    nc.vector.tensor_copy(ob[:], pp[:])
                nc.sync.dma_start(
                    out[tb * P:(tb + 1) * P, oh * TQ:(oh + 1) * TQ], ob[:])

            pending = []  # (segment_stashed, callable)

            def drain(upto_seg):
                while pending and pending[0][0] <= upto_seg:
                    pending.pop(0)[1]()

            # prologue: block 0 qkv + v-transposes for both pairs
            for j in range(2):
                for m in (j, 2 + j, 4 + j):
                    emit_chain(0, m)
                for kt in range(4):
                    emit_pt(j, kt)
                emit_ones(0, j)

            for qb in range(NQB):
                q0 = qb * TQ
                nkt = (qb + 1) * (TQ // P)
                for j in range(2):
                    s = 2 * qb + j
                    drain(s - 2)
                    # stash next-block qkv work for this pair
                    if qb + 1 < NQB:
                        n = qb + 1
                        for m in (j, 2 + j, 4 + j):
                            pending.append((s, (lambda n=n, m=m: emit_chain(n, m))))
                        for kt in range(4 * n, 4 * n + 4):
                            pending.append((s, (lambda j=j, kt=kt: emit_pt(j, kt))))
                        pending.append((s, (lambda n=n, j=j: emit_ones(n, j))))
                    # stash proj for the previous q block (half per pair)
                    if qb >= 1:
                        for tb in range((qb - 1) * 4 + 2 * j,
                                        (qb - 1) * 4 + 2 * j + 2):
                            for oh in range(2):
                                pending.append(
                                    (s, (lambda tb=tb, oh=oh: emit_pp(tb, oh, "dve"))))

                    # ---- attention k-loop for (qb, pair j) ----
                    qm, km = qkvT[j], qkvT[2 + j]
                    avp = [psAv.tile([D + 1, TQ], F32, tag=f"av{hh}",
                                     name=f"av{hh}_{qb}_{j}")
                           for hh in range(2)]
                    for kt in range(nkt):
                        z = max(0, kt * P - q0)
                        sc = psSC.tile([P, 2 * TQ], F32, tag="sc",
                                       name=f"sc_{qb}_{j}_{kt}")
                        scr = sc.rearrange("p (g x) -> p g x", g=2)
                        ee = esb.tile([P, 2 * TQ], F32R, tag="ee",
                                      name=f"ee_{qb}_{j}_{kt}")
                        eer = ee.rearrange("p (g x) -> p g x", g=2)
                        for hh in range(2):
                            nc.tensor.matmul(
                                scr[:, hh, z:],
                                km[hh * D:(hh + 1) * D, kt * P:(kt + 1) * P],
                                qm[hh * D:(hh + 1) * D, q0 + z:q0 + TQ],
                                start=True, stop=True,
                                tile_position=(hh * D, 0),
                            )
                        nc.scalar.activation(
                            eer[:, :, z:], scr[:, :, z:], AF.Exp, scale=0.125)
                        if kt * P >= q0:  # diagonal band: causal mask on POOL
                            nc.gpsimd.affine_select(
                                out=eer[:, :, z:z + P], in_=eer[:, :, z:z + P],
                                compare_op=mybir.AluOpType.is_ge, fill=0.0,
                                base=0, pattern=[[0, 2], [1, P]],
                                channel_multiplier=-1)
                        for hh in range(2):
                            nc.tensor.matmul(
                                avp[hh][:, z:],
                                vrs[j][:, kt, hh, :],
                                eer[:, hh, z:],
                                start=(kt == 0), stop=(kt == nkt - 1),
                            )
                        # pace deferred PE work into the ACT-bound loop
                        iters_left = nkt - kt
                        k = 1 if len(pending) <= iters_left else 2
                        for _ in range(min(k, len(pending))):
                            pending.pop(0)[1]()

                    # softmax division (deferred off the critical path):
                    # yT = avT * (1/sums) broadcast
                    def emit_div(qb=qb, j=j, q0=q0, avp=avp):
                        for hh in range(2):
                            rp = small.tile([1, TQ], F32R, tag="rp",
                                            name=f"rp_{qb}_{j}_{hh}")
                            with nc.allow_low_precision(reason="f32r recip"):
                                nc.vector.reciprocal(rp[:], avp[hh][D:D + 1, :])
                            bc = psMM.tile([D, TQ], F32, tag="mm",
                                           name=f"bc_{qb}_{j}_{hh}")
                            nc.tensor.matmul(bc[:], onesf[0:1, :], rp[:],
                                             start=True, stop=True)
                            bcs = small.tile([D, TQ], F32R, tag="bcs",
                                             name=f"bcs_{qb}_{j}_{hh}")
                            nc.vector.tensor_copy(bcs[:], bc[:])
                            nc.vector.tensor_mul(
                                yT[j][hh * D:(hh + 1) * D, q0:q0 + TQ],
                                avp[hh][:D, :], bcs[:])
                    if qb == NQB - 1:
                        emit_div()
                    else:
                        pending.append((2 * qb + j, emit_div))

            # tail: proj for the last q block (split copies DVE/ACT)
            for tb in range(3 * 4, 4 * 4):
                for oh in range(2):
                    emit_pp(tb, oh, "act" if (tb + oh) % 2 else "dve")


    nc.compile()
    return nc


def _prep_inputs(x, w_qkv, w_proj):
    """Build per-core input maps. Core c = b * 4 + hg."""
    in_maps = []
    xTb = [np.ascontiguousarray(x[b].T) for b in range(B)]
    cst = np.zeros((P, 576), dtype=np.float32)
    cst[:, 0:P] = np.eye(P, dtype=np.float32)
    tri = np.triu(np.ones((P, P), dtype=np.float32))  # 1 where pk <= fq
    cst[:, P:2 * P] = tri
    cst[:, 2 * P:3 * P] = tri
    cst[:, 3 * P:3 * P + D] = 1.0
    cst[0, 3 * P + D:3 * P + D + D] = 1.0        # sel row 0: cols 0:64
    cst[1, 3 * P + 2 * D:3 * P + 3 * D] = 1.0    # sel row 1: cols 64:128
    for b in range(B):
        for hg in range(HG):
            sl = slice(hg * CG, (hg + 1) * CG)
            q, k, v = w_qkv[sl], w_qkv[C:][sl], w_qkv[2 * C:][sl]
            # physical column order: q01 k01 v01 | q23 k23 v23
            wTg = np.ascontiguousarray(np.concatenate(
                [q[:P], k[:P], v[:P], q[P:], k[P:], v[P:]], axis=0).T)
            pTg = np.ascontiguousarray(w_proj[:, sl].T)
            in_maps.append({"xT": xTb[b], "wT": wTg, "pT": pTg, "cst": cst})
    return in_maps


def kernel(x, w_qkv, w_proj):
    x = np.asarray(x, dtype=np.float32)
    w_qkv = np.asarray(w_qkv, dtype=np.float32)
    w_proj = np.asarray(w_proj, dtype=np.float32)

    if "nc" not in _cached:
        _cached["nc"] = _build_nc()
    nc = _cached["nc"]

    in_maps = _prep_inputs(x, w_qkv, w_proj)
    res = bass_utils.run_bass_kernel_spmd(nc, in_maps, core_ids=list(range(N_CORES)))

    out = np.zeros((B, T, C), dtype=np.float32)
    for b in range(B):
        for hg in range(HG):
            out[b] += res.results[b * HG + hg]["out"]
    return out



# revision 45
# speedup vs baseline: 1.2569x; 1.2569x over previous
"""Causal self-attention (B=2, T=2048, C=1024, H=16) on 8 TRN2 NeuronCores.

Sharding: 8 cores = 2 batches x 4 head-groups (4 heads each).
Each core computes qkv for its heads, causal attention, and a partial
output projection; the host sums the 4 partial projections per batch.

All matmuls run in bf16 (1 cycle/row at any output width; f32 PSUM
accumulation). Inputs are converted to bf16 on the host, halving DMA,
and DMAs are merged across contraction tiles with one SBUF tile per
DMA so dependency tracking stays exact while HWDGE descriptor
generation overhead is amortized.

Layouts (per core, pair j in {0,1} = heads (2j, 2j+1)):
  x^T     [128, c, t]   contraction c = partition + tile index
  wq01/wk01/wqk23       qkv weights by m-tile: [q01|k01|q23|k23]
  wvs     [128, 8, 256] v weights, columns [v01|v23]
  qkT     [128, T] x4   q/k feature-major (d on partitions)
  vaug[j] [128, 16, 2, 65] pos-major v per k-tile: [v_h | ones]
  scores in scoresT layout: partition = k, free = (head, q); exp on
  ScalarE covers both heads of a pair in one instruction.
  av^T accumulated via matmul with vaug (ones column -> softmax sums).
  division: reciprocal rows (partition 0 / 32) -> one K=33 broadcast
  matmul -> one [128, TQ] DVE multiply into yT.

The attention k-loop is software-pipelined depth 2 (first two AVs
delayed to iteration 3 so the previous segment's PSUM evacuation never
stalls PE). Deferred qkv-chain / projection work is spread across each
segment's iterations with precomputed assignments so PE stays fed
everywhere, including the last q-block.
"""

import numpy as np
import ml_dtypes

import concourse.bass as bass
import concourse.mybir as mybir
import concourse.tile as tile
from concourse import bacc, bass_utils

F32 = mybir.dt.float32
BF16 = mybir.dt.bfloat16
FP8 = mybir.dt.float8e4
DR = mybir.MatmulPerfMode.DoubleRow
AF = mybir.ActivationFunctionType
BF16NP = ml_dtypes.bfloat16

B = 2
T = 2048
C = 1024
D = 64
N_CORES = 8
HG = 4            # heads per core
CG = HG * D       # 256 y-columns per core
P = 128
TQ = 512          # q block width
NKT = T // P      # 16 k tiles
NQB = T // TQ     # 4 q blocks
NCT = C // P      # 8 contraction tiles for qkv

_cached = {}


def _build_nc():
    nc = bacc.Bacc("TRN2", target_bir_lowering=False, debug=False,
                   num_devices=N_CORES)
    xT = nc.dram_tensor("xT", [C, T], BF16, kind="ExternalInput")
    wqk = nc.dram_tensor("wqk", [C, 4 * P], BF16, kind="ExternalInput")
    wv = nc.dram_tensor("wv", [C, 2 * P], BF16, kind="ExternalInput")
    pT = nc.dram_tensor("pT", [CG, C], BF16, kind="ExternalInput")
    cst = nc.dram_tensor("cst", [33, P], BF16, kind="ExternalInput")
    out = nc.dram_tensor("out", [T, C], BF16, kind="ExternalOutput")

    xTr = xT.rearrange("(a p) t -> p a t", p=P)
    wqkr = wqk.rearrange("(a p) w -> p a w", p=P)
    wvr = wv.rearrange("(a p) w -> p a w", p=P)
    pTr = pT.rearrange("(a p) w -> p a w", p=P)

    with tile.TileContext(nc) as tc:
        with (
            tc.tile_pool(name="const", bufs=1) as const,
            tc.tile_pool(name="persist", bufs=1) as persist,
            tc.tile_pool(name="esb", bufs=5) as esb,
            tc.tile_pool(name="obp", bufs=2) as obp,
            tc.tile_pool(name="bcp", bufs=2) as bcp,
            tc.tile_pool(name="psMM", bufs=2, space="PSUM") as psMM,
            tc.tile_pool(name="psSC", bufs=2, space="PSUM") as psSC,
            tc.tile_pool(name="psAv", bufs=1, space="PSUM") as psAv,
        ):
            # -------- constants (DMA'd after the x/w tensors) --------
            sel = const.tile([33, P], BF16, tag="sel")

            # ---------------- persistent tensors ----------------
            # one SBUF tile per DMA so dependency tracking stays exact;
            # block 0 is split per c-pair so the first qkv chains start as
            # soon as the earliest bytes land
            xb0 = [persist.tile([P, 2 * TQ], BF16, tag=f"xb0{h}",
                                name=f"xb0{h}") for h in range(4)]
            xb0r = [t.rearrange("p (a t) -> p a t", a=2) for t in xb0]
            xbn = [persist.tile([P, NCT * TQ], BF16, tag=f"xb{n}",
                                name=f"xb{n}") for n in range(1, 4)]
            xbnr = [t.rearrange("p (a t) -> p a t", a=NCT) for t in xbn]

            def xslice(c, t0, t1):
                """x^T [c-tile, cols t0:t1] (within one q block)."""
                n = t0 // TQ
                if n == 0:
                    return xb0r[c // 2][:, c % 2, t0:t1]
                return xbnr[n - 1][:, c, t0 - n * TQ:t1 - n * TQ]

            wqk01 = [persist.tile([P, 2 * 2 * P], BF16, tag=f"wqk01{h}",
                                  name=f"wqk01{h}") for h in range(4)]
            wqk01r = [t.rearrange("p (a w) -> p a w", a=2) for t in wqk01]
            wqk23 = persist.tile([P, NCT * 2 * P], BF16, tag="wqk23",
                                 name="wqk23")
            wqk23r = wqk23.rearrange("p (a w) -> p a w", a=NCT)

            def wqk_slice(c, m):
                if m < 2:
                    return wqk01r[c // 2][:, c % 2, m * P:(m + 1) * P]
                return wqk23r[:, c, (m % 2) * P:(m % 2 + 1) * P]

            wvs = persist.tile([P, NCT * 2 * P], BF16, tag="wvs", name="wvs")
            wvsr = wvs.rearrange("p (a w) -> p a w", a=NCT)
            wps = persist.tile([P, 2 * C], BF16, tag="wps", name="wps")
            wpr = wps.rearrange("p (a w) -> p a w", a=2)

            # qk m-tiles: 0=q01 1=k01 2=q23 3=k23
            qkT = [persist.tile([P, T], BF16, tag=f"qkT{m}", name=f"qkT{m}")
                   for m in range(4)]
            # split by k-tile quarter: region-level reuse of one big tile
            # would create false tile-granular WAR dependencies
            vaug = {(j, qt): persist.tile([P, 4 * 2 * (D + 1)], BF16,
                                          tag=f"vaug{j}_{qt}",
                                          name=f"vaug{j}_{qt}")
                    for j in range(2) for qt in range(4)}
            vrs = {k: t.rearrange("p (k g x) -> p k g x", k=4, g=2)
                   for k, t in vaug.items()}

            def vslice(j, kt, hh):
                return vrs[(j, kt // 4)][:, kt % 4, hh, :]
            yT = [persist.tile([P, T], BF16, tag=f"yT{j}", name=f"yT{j}")
                  for j in range(2)]
            # reciprocal rows: head0 at partition 0, head1 at partition 32
            # (engine writes must start at an aligned partition)
            rp2 = persist.tile([33, TQ], BF16, tag="rp2", name="rp2")

            # ---------------- input DMAs ----------------
            # ordered so PE work unlocks as early as possible: the first v
            # chains need only (wv, x block0); sel is not needed until the
            # first division broadcast.
            for h in range(4):
                nc.sync.dma_start(wqk01[h][:],
                                  wqkr[:, 2 * h:2 * h + 2, 0:2 * P])
                nc.sync.dma_start(xb0r[h][:],
                                  xTr[:, 2 * h:2 * h + 2, 0:TQ])
            nc.sync.dma_start(wvs[:], wvr[:])
            nc.sync.dma_start(wqk23[:], wqkr[:, :, 2 * P:])
            nc.sync.dma_start(xbn[0][:], xTr[:, :, TQ:2 * TQ])
            nc.sync.dma_start(xbn[1][:], xTr[:, :, 2 * TQ:3 * TQ])
            nc.sync.dma_start(wps[:], pTr[:])
            nc.sync.dma_start(xbn[2][:], xTr[:, :, 3 * TQ:])
            nc.sync.dma_start(sel[:], cst[:])

            # ones columns of vaug (disjoint from the v copies); zero the
            # unused reciprocal rows once
            for j in range(2):
                for qt in range(4):
                    nc.gpsimd.memset(vrs[(j, qt)][:, :, :, D:D + 1], 1.0)
            nc.gpsimd.memset(rp2[0:32, :], 0.0)

            # ---------------- work items ----------------
            def emit_qk_chain(m, n):
                """qkT[m][:, n*TQ:] = wqk col-block m ^T @ x block n."""
                ps = psMM.tile([P, TQ], F32, tag="mm", name=f"qk_{m}_{n}")
                for c in range(NCT):
                    nc.tensor.matmul(
                        ps[:], wqk_slice(c, m),
                        xslice(c, n * TQ, (n + 1) * TQ),
                        start=(c == 0), stop=(c == NCT - 1))
                # block-0 copies ride on ACT (idle then); later on DVE so
                # they never delay the exp stream
                eng = nc.scalar.copy if n == 0 else nc.vector.tensor_copy
                eng(qkT[m][:, n * TQ:(n + 1) * TQ], ps[:])

            def emit_v(kt, j):
                """vaug[j][:, kt, :, :D] = x^T t-tile kt @ wv pair j."""
                ps = psMM.tile([P, TQ], F32, tag="mm", name=f"v_{kt}_{j}")
                for c in range(NCT):
                    nc.tensor.matmul(
                        ps[:, 0:P], xslice(c, kt * P, (kt + 1) * P),
                        wvsr[:, c, j * P:(j + 1) * P],
                        start=(c == 0), stop=(c == NCT - 1))
                psr = ps[:, 0:P].rearrange("p (g x) -> p g x", g=2)
                # ACT takes the early copies except where it would delay the
                # exp stream (pair-1 k-tiles 0-3 land inside s0/s1)
                eng = (nc.scalar.copy if kt <= 7 and not (j == 1 and kt <= 3)
                       else nc.vector.tensor_copy)
                eng(vrs[(j, kt // 4)][:, kt % 4, :, :D], psr)

            _obs = {}

            def emit_proj(tb, oh):
                """Partial projection for t-tile tb, output half oh."""
                if oh == 0:
                    _obs[tb] = obp.tile([P, 2 * TQ], BF16, tag="ob",
                                        name=f"ob_{tb}")
                ob = _obs[tb]
                pp = psMM.tile([P, TQ], F32, tag="mm", name=f"pp_{tb}_{oh}")
                for cc in range(2):
                    nc.tensor.matmul(
                        pp[:], yT[cc][:, tb * P:(tb + 1) * P],
                        wpr[:, cc, oh * TQ:(oh + 1) * TQ],
                        start=(cc == 0), stop=(cc == 1))
                if oh == 0:
                    nc.vector.tensor_copy(ob[:, 0:TQ], pp[:])
                else:
                    # DVE, not ACT: these run in the last two segments where
                    # ACT's exp stream is the pacer
                    nc.vector.tensor_copy(ob[:, TQ:], pp[:])
                    nc.sync.dma_start(out[tb * P:(tb + 1) * P, :], ob[:])

            def emit_bcast_div(qb, j, avp, tail=False):
                """PE broadcast of reciprocals + DVE muls -> yT. The muls
                read avp straight out of PSUM: they both apply the softmax
                division and evacuate the accumulators for the next segment.
                """
                q0 = qb * TQ
                bc = psMM.tile([P, TQ], F32, tag="mm", name=f"bc_{qb}_{j}")
                nc.tensor.matmul(bc[:], sel[:], rp2[:], start=True, stop=True)
                # DVE can read only one PSUM operand per instruction: stage
                # the broadcast in SBUF, then multiply against avp (PSUM).
                # At the tail ACT is idle and the DVE queue is the critical
                # path, so stage there.
                bcs = bcp.tile([P, TQ], BF16, tag="bcs", name=f"bcs_{qb}_{j}")
                if tail:
                    nc.scalar.copy(bcs[:], bc[:])
                else:
                    nc.vector.tensor_copy(bcs[:], bc[:])
                for hh in range(2):
                    with nc.allow_low_precision(reason="bf16 softmax div"):
                        nc.vector.tensor_mul(
                            yT[j][hh * D:(hh + 1) * D, q0:q0 + TQ],
                            avp[hh][:D, :], bcs[hh * D:(hh + 1) * D, :])

            # ---------------- segment schedule ----------------
            # deferred PE work per segment s = 2*qb + j; each entry must be
            # complete before the data it produces is consumed (qk/v for
            # block n before segment 2n; proj for q-block qb after segment
            # 2qb+1's division).
            work = {s: [] for s in range(8)}

            def W(s, fn):
                work[s].append(fn)

            for kt in range(4):
                W(0, (lambda kt=kt: emit_v(kt, 0)))
            W(0, lambda: emit_qk_chain(2, 0))
            W(0, lambda: emit_qk_chain(3, 0))
            for kt in range(4):
                W(1, (lambda kt=kt: emit_v(kt, 1)))
            for m in range(4):
                W(1, (lambda m=m: emit_qk_chain(m, 1)))
            for kt in range(4, 8):
                W(2, (lambda kt=kt: emit_v(kt, 0)))
                W(2, (lambda kt=kt: emit_v(kt, 1)))
            W(2, lambda: emit_qk_chain(0, 2))
            W(2, lambda: emit_qk_chain(1, 2))
            W(3, lambda: emit_qk_chain(2, 2))
            W(3, lambda: emit_qk_chain(3, 2))
            W(4, lambda: emit_qk_chain(0, 3))
            W(4, lambda: emit_qk_chain(1, 3))
            for kt in range(8, 12):
                W(4, (lambda kt=kt: emit_v(kt, 0)))
                W(4, (lambda kt=kt: emit_v(kt, 1)))
            W(5, lambda: emit_qk_chain(2, 3))
            W(5, lambda: emit_qk_chain(3, 3))
            for kt in range(12, 14):
                W(5, (lambda kt=kt: emit_v(kt, 0)))
                W(5, (lambda kt=kt: emit_v(kt, 1)))
            for kt in range(14, 16):     # used by s6's last AV pairs
                W(6, (lambda kt=kt: emit_v(kt, 0)))
                W(6, (lambda kt=kt: emit_v(kt, 1)))
            for tb in range(4):          # q-block 0 -> s6
                for oh in range(2):
                    W(6, (lambda tb=tb, oh=oh: emit_proj(tb, oh)))
            W(6, lambda: emit_proj(8, 0))
            W(6, lambda: emit_proj(8, 1))
            for tb in range(4, 8):       # q-block 1 -> s7
                for oh in range(2):
                    W(7, (lambda tb=tb, oh=oh: emit_proj(tb, oh)))
            for tb in range(9, 12):      # rest of q-block 2 -> s7
                for oh in range(2):
                    W(7, (lambda tb=tb, oh=oh: emit_proj(tb, oh)))

            # ---------------- prologue ----------------
            # q01/k01 chains c-interleaved so every step starts as soon as
            # its c-pair DMA lands. v chains follow as s0's first filler.
            ps_q = psMM.tile([P, TQ], F32, tag="mm", name="qk_0_0")
            ps_k = psMM.tile([P, TQ], F32, tag="mm", name="qk_1_0")
            for c in range(NCT):
                nc.tensor.matmul(ps_q[:], wqk_slice(c, 0), xslice(c, 0, TQ),
                                 start=(c == 0), stop=(c == NCT - 1))
                nc.tensor.matmul(ps_k[:], wqk_slice(c, 1), xslice(c, 0, TQ),
                                 start=(c == 0), stop=(c == NCT - 1))
            nc.scalar.copy(qkT[0][:, 0:TQ], ps_q[:])
            nc.scalar.copy(qkT[1][:, 0:TQ], ps_k[:])

            # ---------------- attention segments ----------------
            prev_div = [None]  # deferred (bcast+mul) from previous segment
            slots = {}

            def emit_tail_cc0s():
                sct = None
                for idx, tb in enumerate(range(12, 16)):
                    if idx % 2 == 0:
                        sct = psSC.tile([P, 2 * TQ], F32, tag="sc",
                                        name=f"tpp_{tb}")
                    slots[tb] = sct[:, (idx % 2) * TQ:(idx % 2 + 1) * TQ]
                    nc.tensor.matmul(
                        slots[tb], yT[0][:, tb * P:(tb + 1) * P],
                        wpr[:, 0, 0:TQ], start=True, stop=False,
                        skip_group_check=True)

            for s in range(8):
                qb, j = s // 2, s % 2
                q0 = qb * TQ
                nkt = (qb + 1) * (TQ // P)
                qm, km = qkT[2 * j], qkT[2 * j + 1]
                avp = [psAv.tile([D + 1, TQ], F32, tag=f"av{hh}",
                                 name=f"av{hh}_{s}")
                       for hh in range(2)]

                items = list(work[s])
                # spread items over iterations, first at 0, last near the end
                quota = [0] * nkt
                for idx in range(len(items)):
                    quota[idx * (nkt - 1) // max(1, len(items) - 1)] += 1
                qpos = 0

                npair = nkt // 2
                eeps = []

                def emit_av_pair(p, j=j, q0=q0, npair=npair, avp=avp,
                                 eeps=eeps):
                    """fp8 DoubleRow AV over k-tile pair p: v_hi (with
                    ones column for the softmax sums) then v_lo residual,
                    in 256-wide chunks so the moving free dim stays <=512.
                    """
                    # bf16 per-k-tile matmuls; regions narrow
                    # monotonically so each bank keeps a single open
                    # accumulation group
                    for kti in range(2):
                        kt = 2 * p + kti
                        zk = max(0, kt * P - q0)
                        for h in range(2):
                            nc.tensor.matmul(
                                avp[h][:, zk:],
                                vslice(j, kt, h),
                                eeps[kt][:, h, zk:],
                                start=(kt == 0), stop=(kt == nkt - 1))

                # AV(pair p) is scheduled a few iterations after its exps so
                # neither ACT latency nor the previous segment's PSUM
                # evacuation ever stalls PE
                sched = {}
                for p in range(npair):
                    it = nkt if nkt == 4 else (5 if p <= 1 else 2 * p + 3)
                    sched.setdefault(min(it, nkt), []).append(p)

                for i in range(nkt):
                    if i == 1 and prev_div[0] is not None:
                        prev_div[0]()
                        prev_div[0] = None
                    for p in sched.get(i, []):
                        emit_av_pair(p)
                    z = max(0, i * P - q0)
                    ee = esb.tile([P, 2 * TQ], BF16, tag="ee",
                                  name=f"ee_{s}_{i}")
                    eep = ee.rearrange("p (g x) -> p g x", g=2)
                    eeps.append(eep)
                    sc = psSC.tile([P, 2 * TQ], F32, tag="sc",
                                   name=f"sc_{s}_{i}")
                    scr = sc.rearrange("p (g x) -> p g x", g=2)
                    for hh in range(2):
                        nc.tensor.matmul(
                            scr[:, hh, z:],
                            km[hh * D:(hh + 1) * D, i * P:(i + 1) * P],
                            qm[hh * D:(hh + 1) * D, q0 + z:q0 + TQ],
                            start=True, stop=True,
                            tile_position=(hh * D, 0))
                    nc.scalar.activation(
                        eep[:, :, z:], scr[:, :, z:], AF.Exp, scale=0.125)
                    if i * P >= q0:  # diagonal band: causal mask
                        nc.gpsimd.affine_select(
                            out=eep[:, :, z:z + P], in_=eep[:, :, z:z + P],
                            compare_op=mybir.AluOpType.is_ge, fill=0.0,
                            base=0, pattern=[[0, 2], [1, P]],
                            channel_multiplier=-1)
                    # paced deferred work
                    for _ in range(quota[i]):
                        items[qpos]()
                        qpos += 1
                for p in sched.get(nkt, []):
                    emit_av_pair(p)
                while qpos < len(items):
                    items[qpos]()
                    qpos += 1

                # softmax epilogue: reciprocals of the sums rows; the
                # division itself is deferred into the next segment, where
                # its DVE muls also evacuate the avp banks.
                for hh in range(2):
                    with nc.allow_low_precision(reason="bf16 softmax recip"):
                        nc.vector.reciprocal(rp2[32 * hh:32 * hh + 1, :],
                                             avp[hh][D:D + 1, :])
                prev_div[0] = (lambda qb=qb, j=j, avp=avp, **kw:
                               emit_bcast_div(qb, j, avp, **kw))

            # ---------------- tail ----------------
            emit_tail_cc0s()
            prev_div[0](tail=True)
            for pair in ((12, 13), (14, 15)):
                obs = {tb: obp.tile([P, 2 * TQ], BF16, tag="ob",
                                    name=f"ob_{tb}") for tb in pair}
                # both cc1 writes to the shared psSC tile land before any
                # read, avoiding false tile-level WAR stalls
                for tb in pair:
                    nc.tensor.matmul(
                        slots[tb], yT[1][:, tb * P:(tb + 1) * P],
                        wpr[:, 1, 0:TQ], start=False, stop=True,
                        skip_group_check=True)
                for tb in pair:
                    if tb % 2 == 0:
                        nc.vector.tensor_copy(obs[tb][:, 0:TQ], slots[tb])
                    else:
                        nc.scalar.copy(obs[tb][:, 0:TQ], slots[tb])
                for tb in pair:
                    pp = psMM.tile([P, TQ], F32, tag="mm", name=f"tpp1_{tb}")
                    for cc in range(2):
                        nc.tensor.matmul(
                            pp[:], yT[cc][:, tb * P:(tb + 1) * P],
                            wpr[:, cc, TQ:], start=(cc == 0), stop=(cc == 1))
                    eng = nc.scalar.copy if tb % 2 == 0 else nc.vector.tensor_copy
                    eng(obs[tb][:, TQ:], pp[:])
                    nc.sync.dma_start(out[tb * P:(tb + 1) * P, :], obs[tb][:])

    nc.compile()
    return nc


def _prep_inputs(x, w_qkv, w_proj):
    """Build per-core input maps. Core c = b * 4 + hg."""
    in_maps = []
    xTb = [np.ascontiguousarray(x[b].T).astype(BF16NP) for b in range(B)]
    cst = np.zeros((33, P), dtype=BF16NP)
    cst[0, 0:D] = 1.0
    cst[32, D:2 * D] = 1.0
    wq, wk, wv = w_qkv[:C], w_qkv[C:2 * C], w_qkv[2 * C:]
    for b in range(B):
        for hg in range(HG):
            sl = slice(hg * CG, (hg + 1) * CG)
            q, k, v = wq[sl], wk[sl], wv[sl]
            # wqk columns: [q01 | k01 | q23 | k23]
            wqkg = np.ascontiguousarray(np.concatenate(
                [q[:P], k[:P], q[P:], k[P:]], axis=0).T).astype(BF16NP)
            # wv columns: [v01 | v23]
            wvg = np.ascontiguousarray(v.T).astype(BF16NP)
            pTg = np.ascontiguousarray(w_proj[:, sl].T).astype(BF16NP)
            in_maps.append({"xT": xTb[b], "wqk": wqkg, "wv": wvg,
                            "pT": pTg, "cst": cst})
    return in_maps


def kernel(x, w_qkv, w_proj):
    x = np.asarray(x, dtype=np.float32)
    w_qkv = np.asarray(w_qkv, dtype=np.float32)
    w_proj = np.asarray(w_proj, dtype=np.float32)

    if "nc" not in _cached:
        _cached["nc"] = _build_nc()
    nc = _cached["nc"]

    in_maps = _prep_inputs(x, w_qkv, w_proj)
    res = bass_utils.run_bass_kernel_spmd(nc, in_maps,
                                          core_ids=list(range(N_CORES)))

    out = np.zeros((B, T, C), dtype=np.float32)
    for b in range(B):
        for hg in range(HG):
            out[b] += res.results[b * HG + hg]["out"].astype(np.float32)
    return out
